# revision 9
# baseline (speedup 1.0000x reference)
"""GCN autoencoder on 8 TRN2 NeuronCores — PE segment-sum variant.

Differences vs kernel.py (variant A):
  - message passing gathers straight from the AllGathered HBM table with
    plain (non-transpose) dma_gather: slot i lands on partition i%128,
    chunk i//128, as a [128 slot, 128 feat] bf16 block per chunk.
  - the per-destination segment sum is a TensorEngine matmul against a
    static 0/1 selection matrix S (one column per destination node in the
    tile, one row per slot): psum[feat, node] += buf_chunk[slot, feat].T
    @ S_chunk[slot, node]. No bucket padding: each (tile, half) window is
    packed exactly (max over cores, rounded to 128); pad slots point at
    row 0 and S kills them with a zero row. This removes the vector-engine
    reduce and the per-bucket max padding.
  - SWDGE descriptor ring is enlarged (dynamic_dma_scratch_size=32768) so
    each dma_gather call carries 1792 slots; calls rotate over the 4 SWDGE
    queues so descriptor generation overlaps ring drain.

Everything else (node dealing, degree sort, encoder, Shared AllGather,
decoder, host unpermute) matches kernel.py.
"""
import sys

if "/opt/trn_rl_repo" not in sys.path:
    sys.path.insert(0, "/opt/trn_rl_repo")

import numpy as np
import ml_dtypes

import concourse.bacc as bacc
import concourse.bass as bass
import concourse.mybir as mybir
import concourse.tile as tile
from concourse.bass_utils import run_bass_kernel_spmd

NCORES = 8
N = 50000
IN_SIZE = 512
HID = 128
P = 128
REAL = 6250             # real nodes per core
NCN = 6272              # padded nodes per core = 49 * 128
NTILES = NCN // P       # 49
NPAD = NCORES * NCN     # 50176
HALF = NPAD // 2        # 25088 (table half => gather idx < 25088 fits int16)
CALLW = 896             # slots per dma_gather call (2 in flight per ring)
SCRATCH = 32768         # SWDGE descriptor carveout bytes/partition
NQ = 4                  # SWDGE queues: rotated per tile-group (all calls a
                        # consumer waits on share one queue; cross-queue
                        # call-by-call rotation corrupts data on HW)

_cache = {}


def _wrap_idx(arr):
    """int16 index array -> [128, len/16] wrapped layout: slot i at
    [i % 16, i // 16], replicated for the 8 gpsimd cores' partition groups."""
    a = np.asarray(arr, np.int16)
    assert len(a) % 16 == 0
    w = a.reshape(-1, 16).T
    return np.ascontiguousarray(np.tile(w, (8, 1)))


def _build_schedule(edge_index):
    src0 = np.asarray(edge_index[0], np.int64)
    dst0 = np.asarray(edge_index[1], np.int64)

    # global degree sort + round-robin deal: sorted-rank k -> core k%8,
    # local k//8 (pads at shard tails)
    deg_in = np.bincount(dst0, minlength=N) + 1
    order = np.argsort(deg_in, kind="stable")
    core_of = np.empty(N, np.int64)
    local_of = np.empty(N, np.int64)
    core_of[order] = np.arange(N) % NCORES
    local_of[order] = np.arange(N) // NCORES
    node_of = np.full((NCORES, NCN), -1, np.int64)
    for c in range(NCORES):
        node_of[c, :REAL] = order[c::NCORES]

    loops = np.arange(N, dtype=np.int64)
    s0 = np.concatenate([src0, loops])
    d0 = np.concatenate([dst0, loops])
    src = core_of[s0] * NCN + local_of[s0]
    dst = core_of[d0] * NCN + local_of[d0]

    deg = np.bincount(dst, minlength=NPAD)
    dinv = np.zeros(NPAD, np.float32)
    nz = deg > 0
    dinv[nz] = (1.0 / np.sqrt(deg[nz].astype(np.float64))).astype(np.float32)

    core = dst // NCN
    local = dst % NCN
    half = (src >= HALF).astype(np.int64)

    # per (core, local, half) in-edge counts
    cnt = np.bincount((core * NCN + local) * 2 + half,
                      minlength=NPAD * 2).reshape(NCORES, NCN, 2)

    # per-core node order: sort locals by (h0cnt, h1cnt)
    sig = np.zeros((NCORES, NCN), np.int64)      # sig[c, j] = local node
    pos = np.zeros((NCORES, NCN), np.int64)      # pos[c, local] = j
    for c in range(NCORES):
        s = np.lexsort((cnt[c, :, 1], cnt[c, :, 0]))
        sig[c] = s
        pos[c, s] = np.arange(NCN)

    # exact per-(tile,half) windows: width = max over cores, rounded to 128
    cnt_sorted = np.take_along_axis(cnt, sig[:, :, None], axis=1)
    tsum = cnt_sorted.reshape(NCORES, NTILES, P, 2).sum(axis=2)  # [C,T,2]
    wlen = -(-tsum.max(axis=0) // P) * P                          # [T,2]
    assert (wlen > 0).all()

    # slot stream: all h0 windows (t-major), then all h1 windows
    woff = np.zeros((NTILES, 2), np.int64)
    run = 0
    for h in (0, 1):
        for t in range(NTILES):
            woff[t, h] = run
            run += int(wlen[t, h])
    TOT = run                                   # multiple of 128 (and CALLW|128)

    # per-core exclusive prefix of per-node counts within each (tile, half)
    pref = np.zeros((NCORES, NTILES, P, 2), np.int64)
    cs = cnt_sorted.reshape(NCORES, NTILES, P, 2)
    np.cumsum(cs, axis=2, out=pref)
    pref -= cs                                  # exclusive

    gidx = np.zeros((NCORES, TOT), np.int16)    # pad slots -> row 0 (S kills)

    # rank of each edge within its (core, local, half) group
    order_e = np.lexsort((src, half, local, core))
    o_src, o_core, o_local, o_half = (src[order_e], core[order_e],
                                      local[order_e], half[order_e])
    okey = (o_core * NCN + o_local) * 2 + o_half
    gstart = np.concatenate([[0], np.cumsum(np.bincount(
        okey, minlength=NPAD * 2))])[:-1]
    grank = np.arange(len(okey)) - gstart[okey]

    j = pos[o_core, o_local]
    t_, p_ = j // P, j % P
    slot = woff[t_, o_half] + pref[o_core, t_, p_, o_half] + grank
    gidx[o_core, slot] = o_src - o_half * HALF

    # static selection matrix S per core: S[slot, node] = 1 (bf16), laid out
    # [partition = slot%128, (slot//128)*128 + node]
    s_host = np.zeros((NCORES, TOT // P, P, P), ml_dtypes.bfloat16)
    s_host[o_core, slot // P, slot % P, p_] = 1.0
    s_host = np.ascontiguousarray(
        s_host.transpose(0, 2, 1, 3).reshape(NCORES, P, TOT))

    return {
        "dinv": dinv, "wlen": wlen, "woff": woff, "TOT": TOT,
        "gidx": gidx, "sig": sig, "node_of": node_of, "s_host": s_host,
    }


def _build_nc(sched, repeat=1):
    wlen, woff, TOT = sched["wlen"], sched["woff"], sched["TOT"]

    nc = bacc.Bacc("TRN2", target_bir_lowering=False, debug=False,
                   num_devices=NCORES, num_swdge_queues=NQ,
                   dynamic_dma_scratch_size=SCRATCH)
    f32, bf16, i16 = mybir.dt.float32, mybir.dt.bfloat16, mybir.dt.int16

    xT = nc.dram_tensor("xT", [IN_SIZE, NCN], bf16, kind="ExternalInput")
    w_enc = nc.dram_tensor("w_enc", [IN_SIZE, HID], bf16, kind="ExternalInput")
    b_enc = nc.dram_tensor("b_enc", [1, HID], bf16, kind="ExternalInput")
    w_dec = nc.dram_tensor("w_dec", [HID, IN_SIZE], bf16, kind="ExternalInput")
    b_dec = nc.dram_tensor("b_dec", [1, IN_SIZE], bf16, kind="ExternalInput")
    dinv_e = nc.dram_tensor("dinv_e", [P, NTILES], f32, kind="ExternalInput")
    dinvb = nc.dram_tensor("dinvb", [P, NCN], f32, kind="ExternalInput")
    g_d = nc.dram_tensor("gidx", [P, TOT // 16], i16, kind="ExternalInput")
    s_d = nc.dram_tensor("smat", [P, TOT], bf16, kind="ExternalInput")
    out = nc.dram_tensor("out", [NCN, IN_SIZE], f32, kind="ExternalOutput")

    GRP = 3
    # per-(group,half) slot spans
    spans = []
    for g0 in range(0, NTILES, GRP):
        ts = list(range(g0, min(g0 + GRP, NTILES)))
        spans.append([(int(woff[ts[0], h]),
                       int(woff[ts[-1], h] + wlen[ts[-1], h])) for h in (0, 1)])
    gmax = [max(s[h][1] - s[h][0] for s in spans) for h in (0, 1)]
    _gq = [0]

    with tile.TileContext(nc) as tc:
        with (
            tc.tile_pool(name="const", bufs=1) as cp,
            tc.tile_pool(name="dram", bufs=1, space="DRAM") as dram,
            tc.tile_pool(name="psum", bufs=2, space="PSUM") as pp,
        ):
            # ---- constants ----
            ones = cp.tile([1, P], bf16)
            nc.vector.memset(ones[:], 1.0)
            benc_sb = cp.tile([1, HID], bf16)
            nc.sync.dma_start(benc_sb[:], b_enc[:])
            bdec_sb = cp.tile([1, IN_SIZE], bf16)
            nc.sync.dma_start(bdec_sb[:], b_dec[:])
            wdec_sb = cp.tile([HID, IN_SIZE], bf16)
            nc.sync.dma_start(wdec_sb[:], w_dec[:])
            dinv_e_sb = cp.tile([P, NTILES], f32)
            nc.sync.dma_start(dinv_e_sb[:], dinv_e[:])
            dinvb_sb = cp.tile([P, NCN], f32)
            nc.sync.dma_start(dinvb_sb[:], dinvb[:])

            enc_loc = dram.tile([NCN, HID], bf16)
            enc_all = dram.tile([NPAD, HID], bf16)

            for _rep in range(repeat):
                # ---- phase 1: encoder ----
                with tc.tile_pool(name="ph1", bufs=1) as p1, \
                     tc.tile_pool(name="ph1db", bufs=3) as p1db:
                    wenc_sb = p1.tile([P, 4, HID], bf16)
                    for k in range(4):
                        nc.sync.dma_start(wenc_sb[:, k, :],
                                          w_enc[k * P:(k + 1) * P, :])
                    xt_sb = p1.tile([P, 4, NCN], bf16)
                    for k in range(4):
                        nc.sync.dma_start(xt_sb[:, k, :],
                                          xT[k * P:(k + 1) * P, :])
                    for t in range(NTILES):
                        ps = pp.tile([P, HID], f32, tag="ps_enc")
                        nc.tensor.matmul(ps[:], ones[:1, :], benc_sb[:1, :],
                                         start=True, stop=False)
                        for k in range(4):
                            nc.tensor.matmul(
                                ps[:], xt_sb[:, k, t * P:(t + 1) * P],
                                wenc_sb[:, k, :], start=False, stop=(k == 3))
                        enc_t = p1db.tile([P, HID], bf16, tag="enc_t")
                        nc.scalar.activation(enc_t[:], ps[:],
                                             mybir.ActivationFunctionType.Relu,
                                             scale=dinv_e_sb[:, t:t + 1])
                        nc.sync.dma_start(enc_loc[t * P:(t + 1) * P, :], enc_t[:])

                # ---- phase 2: allgather ----
                nc.gpsimd.collective_compute(
                    "AllGather", mybir.AluOpType.bypass,
                    replica_groups=[list(range(NCORES))],
                    ins=[enc_loc.opt()], outs=[enc_all.opt()],
                )

                # ---- phase 3+4: gather, matmul segment-sum, decode ----
                with tc.tile_pool(name="ph3", bufs=1) as p3, \
                     tc.tile_pool(name="gb", bufs=4) as gbp, \
                     tc.tile_pool(name="ph4", bufs=3) as p4:
                    g_sb = p3.tile([P, TOT // 16], i16)
                    nc.sync.dma_start(g_sb[:], g_d[:])

                    for gi, g0 in enumerate(range(0, NTILES, GRP)):
                        grp = list(range(g0, min(g0 + GRP, NTILES)))
                        bufs, smats, bases = {}, {}, {}
                        for h in (0, 1):
                            base, end = spans[gi][h]
                            gcols = end - base
                            bases[h] = base
                            buf = gbp.tile([P, gmax[h] // P, P], bf16,
                                           tag=f"gbuf{h}")
                            bufs[h] = buf
                            smat = gbp.tile([P, 1, gmax[h]], bf16,
                                            tag=f"smat{h}")
                            smats[h] = smat
                            nc.sync.dma_start(smat[:, 0, :gcols],
                                              s_d[:, base:end])
                            src_h = enc_all[h * HALF:(h + 1) * HALF, :]
                            for a in range(0, gcols, CALLW):
                                nw = min(CALLW, gcols - a)
                                nc.gpsimd.dma_gather(
                                    buf[:, a // P:(a + nw) // P, :], src_h,
                                    g_sb[:, (base + a) // 16:
                                         (base + a + nw) // 16],
                                    nw, nw, HID,
                                    queue_num=gi % NQ)
                        for t in grp:
                            ps_agg = pp.tile([P, P], f32, tag="ps_agg")
                            first = True
                            for h in (0, 1):
                                coff = (int(woff[t, h]) - bases[h]) // P
                                cw = int(wlen[t, h]) // P
                                for c in range(cw):
                                    last = (h == 1 and c == cw - 1)
                                    nc.tensor.matmul(
                                        ps_agg[:],
                                        bufs[h][:, coff + c, :],
                                        smats[h][:, 0, (coff + c) * P:
                                                 (coff + c + 1) * P],
                                        start=first, stop=last)
                                    first = False

                            aggT = p4.tile([P, P], bf16, tag="aggT")
                            nc.vector.tensor_mul(
                                aggT[:], ps_agg[:],
                                dinvb_sb[:, t * P:(t + 1) * P])
                            ps = pp.tile([P, IN_SIZE], f32, tag="ps_dec")
                            nc.tensor.matmul(ps[:], ones[:1, :],
                                             bdec_sb[:1, :],
                                             start=True, stop=False)
                            nc.tensor.matmul(ps[:], aggT[:], wdec_sb[:],
                                             start=False, stop=True)
                            o_t = p4.tile([P, IN_SIZE], f32, tag="o_t")
                            nc.scalar.activation(
                                o_t[:], ps[:],
                                mybir.ActivationFunctionType.Sigmoid)
                            nc.sync.dma_start(out[t * P:(t + 1) * P, :],
                                              o_t[:])

    nc.compile()
    return nc


def _prepare(x, W_enc, b_enc, W_dec, b_dec, gcn_bias, edge_index):
    sched = _build_schedule(edge_index)
    dinv, sig, node_of = sched["dinv"], sched["sig"], sched["node_of"]

    x = np.asarray(x, np.float32)
    b_dec_eff = (np.asarray(gcn_bias, np.float32) @
                 np.asarray(W_dec, np.float32) +
                 np.asarray(b_dec, np.float32))

    in_maps = []
    for c in range(NCORES):
        xc = np.zeros((NCN, IN_SIZE), np.float32)
        xc[:REAL] = x[node_of[c, :REAL]]
        xT_c = np.ascontiguousarray(xc.T.astype(ml_dtypes.bfloat16))
        dv = dinv[c * NCN:(c + 1) * NCN]
        dinv_e_c = np.ascontiguousarray(
            dv.reshape(NTILES, P).T.astype(np.float32))
        dinvb_c = np.ascontiguousarray(
            np.tile(dv[sig[c]][None, :], (P, 1)).astype(np.float32))
        in_maps.append({
            "xT": xT_c,
            "w_enc": np.asarray(W_enc, np.float32).astype(ml_dtypes.bfloat16),
            "b_enc": np.asarray(b_enc, np.float32).reshape(1, -1)
                       .astype(ml_dtypes.bfloat16),
            "w_dec": np.asarray(W_dec, np.float32).astype(ml_dtypes.bfloat16),
            "b_dec": b_dec_eff.reshape(1, -1).astype(ml_dtypes.bfloat16),
            "dinv_e": dinv_e_c,
            "dinvb": dinvb_c,
            "gidx": _wrap_idx(sched["gidx"][c]),
            "smat": sched["s_host"][c],
        })
    return sched, in_maps


def kernel(x, W_enc, b_enc, W_dec, b_dec, gcn_bias, edge_index,
           _profile=False):
    key = hash(np.asarray(edge_index).tobytes())
    sched, in_maps = _prepare(x, W_enc, b_enc, W_dec, b_dec, gcn_bias,
                              edge_index)
    if key in _cache:
        nc = _cache[key]
    else:
        nc = _build_nc(sched)
        _cache[key] = nc

    res = run_bass_kernel_spmd(nc, in_maps, core_ids=list(range(NCORES)),
                               trace=_profile)
    sig, node_of = sched["sig"], sched["node_of"]
    outp = np.empty((N, IN_SIZE), np.float32)
    for c in range(NCORES):
        o = res.results[c]["out"]          # row j = node sig[c, j]
        mask = sig[c] < REAL
        outp[node_of[c, sig[c][mask]]] = o[mask]
    if _profile:
        return outp, res
    return outp


# revision 12
# speedup vs baseline: 3.5245x; 3.5245x over previous
"""GCN autoencoder on 8 TRN2 NeuronCores — PE segment-sum variant.

Differences vs kernel.py (variant A):
  - message passing gathers straight from the AllGathered HBM table with
    plain (non-transpose) dma_gather: slot i lands on partition i%128,
    chunk i//128, as a [128 slot, 128 feat] bf16 block per chunk.
  - the per-destination segment sum is a TensorEngine matmul against a
    static 0/1 selection matrix S (one column per destination node in the
    tile, one row per slot): psum[feat, node] += buf_chunk[slot, feat].T
    @ S_chunk[slot, node]. No bucket padding: each (tile, half) window is
    packed exactly (max over cores, rounded to 128); pad slots point at
    row 0 and S kills them with a zero row. This removes the vector-engine
    reduce and the per-bucket max padding.
  - SWDGE descriptor ring is enlarged (dynamic_dma_scratch_size=32768) so
    each dma_gather call carries 1792 slots; calls rotate over the 4 SWDGE
    queues so descriptor generation overlaps ring drain.

Everything else (node dealing, degree sort, encoder, Shared AllGather,
decoder, host unpermute) matches kernel.py.
"""
import sys

if "/opt/trn_rl_repo" not in sys.path:
    sys.path.insert(0, "/opt/trn_rl_repo")

import numpy as np
import ml_dtypes

import concourse.bacc as bacc
import concourse.bass as bass
import concourse.mybir as mybir
import concourse.tile as tile
from concourse.bass_utils import run_bass_kernel_spmd

NCORES = 8
N = 50000
IN_SIZE = 512
HID = 128
P = 128
REAL = 6250             # real nodes per core
NCN = 6272              # padded nodes per core = 49 * 128
NTILES = NCN // P       # 49
NPAD = NCORES * NCN     # 50176
HALF = NPAD // 2        # 25088 (table half => gather idx < 25088 fits int16)
CALLW = 896             # slots per dma_gather call (2 in flight per ring)
SCRATCH = 32768         # SWDGE descriptor carveout bytes/partition
NQ = 4                  # SWDGE queues: rotated per tile-group (all calls a
                        # consumer waits on share one queue; cross-queue
                        # call-by-call rotation corrupts data on HW)

_cache = {}


def _wrap_idx(arr):
    """int16 index array -> [128, len/16] wrapped layout: slot i at
    [i % 16, i // 16], replicated for the 8 gpsimd cores' partition groups."""
    a = np.asarray(arr, np.int16)
    assert len(a) % 16 == 0
    w = a.reshape(-1, 16).T
    return np.ascontiguousarray(np.tile(w, (8, 1)))


def _build_schedule(edge_index):
    src0 = np.asarray(edge_index[0], np.int64)
    dst0 = np.asarray(edge_index[1], np.int64)

    # global degree sort + round-robin deal: sorted-rank k -> core k%8,
    # local k//8 (pads at shard tails)
    deg_in = np.bincount(dst0, minlength=N) + 1
    order = np.argsort(deg_in, kind="stable")
    core_of = np.empty(N, np.int64)
    local_of = np.empty(N, np.int64)
    core_of[order] = np.arange(N) % NCORES
    local_of[order] = np.arange(N) // NCORES
    node_of = np.full((NCORES, NCN), -1, np.int64)
    for c in range(NCORES):
        node_of[c, :REAL] = order[c::NCORES]

    loops = np.arange(N, dtype=np.int64)
    s0 = np.concatenate([src0, loops])
    d0 = np.concatenate([dst0, loops])
    src = core_of[s0] * NCN + local_of[s0]
    dst = core_of[d0] * NCN + local_of[d0]

    deg = np.bincount(dst, minlength=NPAD)
    dinv = np.zeros(NPAD, np.float32)
    nz = deg > 0
    dinv[nz] = (1.0 / np.sqrt(deg[nz].astype(np.float64))).astype(np.float32)

    core = dst // NCN
    local = dst % NCN
    half = (src >= HALF).astype(np.int64)

    # per (core, local, half) in-edge counts
    cnt = np.bincount((core * NCN + local) * 2 + half,
                      minlength=NPAD * 2).reshape(NCORES, NCN, 2)

    # per-core node order: sort locals by (h0cnt, h1cnt)
    sig = np.zeros((NCORES, NCN), np.int64)      # sig[c, j] = local node
    pos = np.zeros((NCORES, NCN), np.int64)      # pos[c, local] = j
    for c in range(NCORES):
        s = np.lexsort((cnt[c, :, 1], cnt[c, :, 0]))
        sig[c] = s
        pos[c, s] = np.arange(NCN)

    # exact per-(tile,half) windows: width = max over cores, rounded to 128
    cnt_sorted = np.take_along_axis(cnt, sig[:, :, None], axis=1)
    tsum = cnt_sorted.reshape(NCORES, NTILES, P, 2).sum(axis=2)  # [C,T,2]
    wlen = -(-tsum.max(axis=0) // P) * P                          # [T,2]
    assert (wlen > 0).all()

    # slot stream: all h0 windows (t-major), then all h1 windows
    woff = np.zeros((NTILES, 2), np.int64)
    run = 0
    for h in (0, 1):
        for t in range(NTILES):
            woff[t, h] = run
            run += int(wlen[t, h])
    TOT = run                                   # multiple of 128 (and CALLW|128)

    # per-core exclusive prefix of per-node counts within each (tile, half)
    pref = np.zeros((NCORES, NTILES, P, 2), np.int64)
    cs = cnt_sorted.reshape(NCORES, NTILES, P, 2)
    np.cumsum(cs, axis=2, out=pref)
    pref -= cs                                  # exclusive

    gidx = np.zeros((NCORES, TOT), np.int16)    # pad slots -> row 0 (S kills)

    # rank of each edge within its (core, local, half) group
    order_e = np.lexsort((src, half, local, core))
    o_src, o_core, o_local, o_half = (src[order_e], core[order_e],
                                      local[order_e], half[order_e])
    okey = (o_core * NCN + o_local) * 2 + o_half
    gstart = np.concatenate([[0], np.cumsum(np.bincount(
        okey, minlength=NPAD * 2))])[:-1]
    grank = np.arange(len(okey)) - gstart[okey]

    j = pos[o_core, o_local]
    t_, p_ = j // P, j % P
    slot = woff[t_, o_half] + pref[o_core, t_, p_, o_half] + grank
    gidx[o_core, slot] = o_src - o_half * HALF

    # static selection matrix S per core: S[slot, node] = 1 (bf16), laid out
    # [partition = slot%128, (slot//128)*128 + node]
    s_host = np.zeros((NCORES, TOT // P, P, P), ml_dtypes.bfloat16)
    s_host[o_core, slot // P, slot % P, p_] = 1.0
    s_host = np.ascontiguousarray(
        s_host.transpose(0, 2, 1, 3).reshape(NCORES, P, TOT))

    return {
        "dinv": dinv, "wlen": wlen, "woff": woff, "TOT": TOT,
        "gidx": gidx, "sig": sig, "node_of": node_of, "s_host": s_host,
    }


def _build_nc(sched, repeat=1):
    wlen, woff, TOT = sched["wlen"], sched["woff"], sched["TOT"]

    nc = bacc.Bacc("TRN2", target_bir_lowering=False, debug=False,
                   num_devices=NCORES, num_swdge_queues=NQ,
                   dynamic_dma_scratch_size=SCRATCH)
    f32, bf16, i16 = mybir.dt.float32, mybir.dt.bfloat16, mybir.dt.int16

    xT = nc.dram_tensor("xT", [IN_SIZE, NCN], bf16, kind="ExternalInput")
    w_enc = nc.dram_tensor("w_enc", [IN_SIZE, HID], bf16, kind="ExternalInput")
    b_enc = nc.dram_tensor("b_enc", [1, HID], bf16, kind="ExternalInput")
    w_dec = nc.dram_tensor("w_dec", [HID, IN_SIZE], bf16, kind="ExternalInput")
    b_dec = nc.dram_tensor("b_dec", [1, IN_SIZE], bf16, kind="ExternalInput")
    dinv_e = nc.dram_tensor("dinv_e", [P, NTILES], f32, kind="ExternalInput")
    dinvb = nc.dram_tensor("dinvb", [P, NCN], f32, kind="ExternalInput")
    g_d = nc.dram_tensor("gidx", [P, TOT // 16], i16, kind="ExternalInput")
    s_d = nc.dram_tensor("smat", [P, TOT], bf16, kind="ExternalInput")
    out = nc.dram_tensor("out", [NCN, IN_SIZE], f32, kind="ExternalOutput")

    GRP = 3
    # per-(group,half) slot spans
    spans = []
    for g0 in range(0, NTILES, GRP):
        ts = list(range(g0, min(g0 + GRP, NTILES)))
        spans.append([(int(woff[ts[0], h]),
                       int(woff[ts[-1], h] + wlen[ts[-1], h])) for h in (0, 1)])
    gmax = [max(s[h][1] - s[h][0] for s in spans) for h in (0, 1)]
    _gq = [0]

    with tile.TileContext(nc) as tc:
        with (
            tc.tile_pool(name="const", bufs=1) as cp,
            tc.tile_pool(name="dram", bufs=1, space="DRAM") as dram,
            tc.tile_pool(name="psum", bufs=2, space="PSUM") as pp,
        ):
            # ---- constants ----
            ones = cp.tile([1, P], bf16)
            nc.vector.memset(ones[:], 1.0)
            benc_sb = cp.tile([1, HID], bf16)
            nc.sync.dma_start(benc_sb[:], b_enc[:])
            bdec_sb = cp.tile([1, IN_SIZE], bf16)
            nc.sync.dma_start(bdec_sb[:], b_dec[:])
            wdec_sb = cp.tile([HID, IN_SIZE], bf16)
            nc.sync.dma_start(wdec_sb[:], w_dec[:])
            dinv_e_sb = cp.tile([P, NTILES], f32)
            nc.sync.dma_start(dinv_e_sb[:], dinv_e[:])
            dinvb_sb = cp.tile([P, NCN], f32)
            nc.sync.dma_start(dinvb_sb[:], dinvb[:])

            enc_loc = dram.tile([NCN, HID], bf16)

            for _rep in range(repeat):
                # per-rep output table: rep k+1's AllGather write must not
                # WAR-serialize against rep k's gather reads
                enc_all = dram.tile([NPAD, HID], bf16,
                                    name=f"enc_all_r{_rep}")
                # ---- phase 1: encoder ----
                with tc.tile_pool(name="ph1", bufs=1) as p1, \
                     tc.tile_pool(name="ph1db", bufs=3) as p1db:
                    wenc_sb = p1.tile([P, 4, HID], bf16)
                    for k in range(4):
                        nc.sync.dma_start(wenc_sb[:, k, :],
                                          w_enc[k * P:(k + 1) * P, :])
                    xt_sb = p1.tile([P, 4, NCN], bf16)
                    for k in range(4):
                        nc.sync.dma_start(xt_sb[:, k, :],
                                          xT[k * P:(k + 1) * P, :])
                    for t in range(NTILES):
                        ps = pp.tile([P, HID], f32, tag="ps_enc")
                        nc.tensor.matmul(ps[:], ones[:1, :], benc_sb[:1, :],
                                         start=True, stop=False)
                        for k in range(4):
                            nc.tensor.matmul(
                                ps[:], xt_sb[:, k, t * P:(t + 1) * P],
                                wenc_sb[:, k, :], start=False, stop=(k == 3))
                        enc_t = p1db.tile([P, HID], bf16, tag="enc_t")
                        nc.scalar.activation(enc_t[:], ps[:],
                                             mybir.ActivationFunctionType.Relu,
                                             scale=dinv_e_sb[:, t:t + 1])
                        nc.sync.dma_start(enc_loc[t * P:(t + 1) * P, :], enc_t[:])

                # ---- phase 2: allgather ----
                nc.gpsimd.collective_compute(
                    "AllGather", mybir.AluOpType.bypass,
                    replica_groups=[list(range(NCORES))],
                    ins=[enc_loc.opt()], outs=[enc_all.opt()],
                )

                # ---- phase 3+4: gather, matmul segment-sum, decode ----
                with tc.tile_pool(name="ph3", bufs=1) as p3, \
                     tc.tile_pool(name="gb", bufs=4) as gbp, \
                     tc.tile_pool(name="ph4", bufs=3) as p4:
                    g_sb = p3.tile([P, TOT // 16], i16)
                    nc.sync.dma_start(g_sb[:], g_d[:])

                    for gi, g0 in enumerate(range(0, NTILES, GRP)):
                        grp = list(range(g0, min(g0 + GRP, NTILES)))
                        bufs, smats, bases = {}, {}, {}
                        for h in (0, 1):
                            base, end = spans[gi][h]
                            gcols = end - base
                            bases[h] = base
                            buf = gbp.tile([P, gmax[h] // P, P], bf16,
                                           tag=f"gbuf{h}")
                            bufs[h] = buf
                            smat = gbp.tile([P, 1, gmax[h]], bf16,
                                            tag=f"smat{h}")
                            smats[h] = smat
                            nc.sync.dma_start(smat[:, 0, :gcols],
                                              s_d[:, base:end])
                            src_h = enc_all[h * HALF:(h + 1) * HALF, :]
                            for a in range(0, gcols, CALLW):
                                nw = min(CALLW, gcols - a)
                                nc.gpsimd.dma_gather(
                                    buf[:, a // P:(a + nw) // P, :], src_h,
                                    g_sb[:, (base + a) // 16:
                                         (base + a + nw) // 16],
                                    nw, nw, HID,
                                    queue_num=gi % NQ)
                        for t in grp:
                            ps_agg = pp.tile([P, P], f32, tag="ps_agg")
                            first = True
                            for h in (0, 1):
                                coff = (int(woff[t, h]) - bases[h]) // P
                                cw = int(wlen[t, h]) // P
                                for c in range(cw):
                                    last = (h == 1 and c == cw - 1)
                                    nc.tensor.matmul(
                                        ps_agg[:],
                                        bufs[h][:, coff + c, :],
                                        smats[h][:, 0, (coff + c) * P:
                                                 (coff + c + 1) * P],
                                        start=first, stop=last)
                                    first = False

                            aggT = p4.tile([P, P], bf16, tag="aggT")
                            nc.vector.tensor_mul(
                                aggT[:], ps_agg[:],
                                dinvb_sb[:, t * P:(t + 1) * P])
                            ps = pp.tile([P, IN_SIZE], f32, tag="ps_dec")
                            nc.tensor.matmul(ps[:], ones[:1, :],
                                             bdec_sb[:1, :],
                                             start=True, stop=False)
                            nc.tensor.matmul(ps[:], aggT[:], wdec_sb[:],
                                             start=False, stop=True)
                            o_t = p4.tile([P, IN_SIZE], f32, tag="o_t")
                            nc.scalar.activation(
                                o_t[:], ps[:],
                                mybir.ActivationFunctionType.Sigmoid)
                            nc.sync.dma_start(out[t * P:(t + 1) * P, :],
                                              o_t[:])

    nc.compile()
    return nc


def _prepare(x, W_enc, b_enc, W_dec, b_dec, gcn_bias, edge_index):
    sched = _build_schedule(edge_index)
    dinv, sig, node_of = sched["dinv"], sched["sig"], sched["node_of"]

    x = np.asarray(x, np.float32)
    b_dec_eff = (np.asarray(gcn_bias, np.float32) @
                 np.asarray(W_dec, np.float32) +
                 np.asarray(b_dec, np.float32))

    in_maps = []
    for c in range(NCORES):
        xc = np.zeros((NCN, IN_SIZE), np.float32)
        xc[:REAL] = x[node_of[c, :REAL]]
        xT_c = np.ascontiguousarray(xc.T.astype(ml_dtypes.bfloat16))
        dv = dinv[c * NCN:(c + 1) * NCN]
        dinv_e_c = np.ascontiguousarray(
            dv.reshape(NTILES, P).T.astype(np.float32))
        dinvb_c = np.ascontiguousarray(
            np.tile(dv[sig[c]][None, :], (P, 1)).astype(np.float32))
        in_maps.append({
            "xT": xT_c,
            "w_enc": np.asarray(W_enc, np.float32).astype(ml_dtypes.bfloat16),
            "b_enc": np.asarray(b_enc, np.float32).reshape(1, -1)
                       .astype(ml_dtypes.bfloat16),
            "w_dec": np.asarray(W_dec, np.float32).astype(ml_dtypes.bfloat16),
            "b_dec": b_dec_eff.reshape(1, -1).astype(ml_dtypes.bfloat16),
            "dinv_e": dinv_e_c,
            "dinvb": dinvb_c,
            "gidx": _wrap_idx(sched["gidx"][c]),
            "smat": sched["s_host"][c],
        })
    return sched, in_maps


def kernel(x, W_enc, b_enc, W_dec, b_dec, gcn_bias, edge_index,
           _profile=False):
    key = hash(np.asarray(edge_index).tobytes())
    sched, in_maps = _prepare(x, W_enc, b_enc, W_dec, b_dec, gcn_bias,
                              edge_index)
    if key in _cache:
        nc = _cache[key]
    else:
        nc = _build_nc(sched)
        _cache[key] = nc

    res = run_bass_kernel_spmd(nc, in_maps, core_ids=list(range(NCORES)),
                               trace=_profile)
    sig, node_of = sched["sig"], sched["node_of"]
    outp = np.empty((N, IN_SIZE), np.float32)
    for c in range(NCORES):
        o = res.results[c]["out"]          # row j = node sig[c, j]
        mask = sig[c] < REAL
        outp[node_of[c, sig[c][mask]]] = o[mask]
    if _profile:
        return outp, res
    return outp
